# revision 17
# baseline (speedup 1.0000x reference)
"""Trainium2 Bass kernel for the GeneticAlgorithm step.

Computation (per population pair i, i+N/2):
  crossover: swap cols [s_i, s_i+seg) between the two rows
  stats:     per-row mean / min / max of the crossed matrix
  mutation:  out = where(u_mask < 0.01, clip(crossed + u_noise*avg, mn, mx), crossed)

Key rewrite: since mn <= crossed <= mx per row, clip(crossed, mn, mx) == crossed,
so  out = clip(crossed + (u_mask < 0.01) * u_noise * avg, mn, mx)  exactly.

Sharding: data-parallel over 8 cores; core c owns pairs [256c, 256c+256).
Top/bottom partner rows are co-resident, all reductions are per-row, so there
is no cross-core communication.

Engine plan per [128, 2048] chunk (both halves):
  window mask via integer trick  y = min(max(iota, slo), shi-1);
  mask = (y == iota)  -- exact in fp16 since equality only fires for
  iota in [0, 2047], all fp16-exact.
    DVE:  y (tensor_scalar 4x f16), d2 = mask*d (TT f16 2x),
          crossed halves via STT +/-d2 with free row-sum accums,
          max/min row reduces, and in pass 2 two TS (2x) + two f16 TT (2x)
          for (um<rate)*avg, *u_noise, +crossed, clip/widen.
    POOL: mask equality TT and the f32 pair diff d = bot - top (idle engine).
    ACT:  u_noise f32->f16 casts (idle engine).
fp16 intermediates cost ~3e-3 abs error vs ~5.4 data range; harness gate is
rel 2e-2 (~0.11 abs).
"""

import numpy as np

import concourse.bass as bass
import concourse.bacc as bacc
import concourse.mybir as mybir
from concourse.bass_utils import run_bass_kernel_spmd
from concourse.tile import TileContext

# Problem geometry (hardcoded per spec).
N = 4096           # population size
L = 16384          # genes per individual
HALF = N // 2      # 2048 pairs
NCORES = 8
PPC = HALF // NCORES   # 256 pairs per core
P = 128                # partitions
BLOCKS = PPC // P      # 2 blocks of 128 pairs per core
C = 2048               # column chunk
NCH = L // C           # chunks per row
MUTATION_RATE = 0.01
MASK_ON_GP = False     # Pool ISA check rejects f16 is_equal TT
D_ON_GP = True         # pair diff TT on the Pool engine

F32 = mybir.dt.float32
F16 = mybir.dt.float16
X = mybir.AxisListType.X
OP = mybir.AluOpType
ACT = mybir.ActivationFunctionType

_NC_CACHE = {}


def _build_program():
    nc = bacc.Bacc()

    top = nc.dram_tensor("top", [PPC, L], F32, kind="ExternalInput")
    bot = nc.dram_tensor("bot", [PPC, L], F32, kind="ExternalInput")
    un_top = nc.dram_tensor("un_top", [PPC, L], F32, kind="ExternalInput")
    un_bot = nc.dram_tensor("un_bot", [PPC, L], F32, kind="ExternalInput")
    um_top = nc.dram_tensor("um_top", [PPC, L], F32, kind="ExternalInput")
    um_bot = nc.dram_tensor("um_bot", [PPC, L], F32, kind="ExternalInput")
    # Per-chunk-adjusted crossover bounds: slo_adj[b,p,j] = s - C*j,
    # shim1_adj[b,p,j] = s + seg - 1 - C*j  (f32; exact for values < 2^24).
    slo_adj = nc.dram_tensor("slo_adj", [BLOCKS, P, NCH], F32, kind="ExternalInput")
    shim1_adj = nc.dram_tensor("shim1_adj", [BLOCKS, P, NCH], F32,
                               kind="ExternalInput")
    iota_in = nc.dram_tensor("iota_in", [P, C], F16, kind="ExternalInput")

    out_top = nc.dram_tensor("out_top", [PPC, L], F32, kind="ExternalOutput")
    out_bot = nc.dram_tensor("out_bot", [PPC, L], F32, kind="ExternalOutput")

    with TileContext(nc) as tc:
        with (
            tc.tile_pool(name="const", bufs=1) as const_pool,
            tc.tile_pool(name="popc", bufs=NCH) as pop_pool,
            tc.tile_pool(name="stage", bufs=2) as stage_pool,
            tc.tile_pool(name="scratch", bufs=1) as scratch_pool,
            tc.tile_pool(name="stream", bufs=2) as stream_pool,
            tc.tile_pool(name="p2tmp", bufs=1) as p2_pool,
            tc.tile_pool(name="outp", bufs=2) as out_pool,
            tc.tile_pool(name="stats", bufs=2) as stats_pool,
        ):
            iota_t = const_pool.tile([P, C], F16)
            nc.sync.dma_start(iota_t[:], iota_in[:])

            st = {}  # per-block tile state

            def start_block(b):
                slo_t = stats_pool.tile([P, NCH], F32, tag="slo", name=f"slo{b}")
                shi_t = stats_pool.tile([P, NCH], F32, tag="shi", name=f"shi{b}")
                nc.sync.dma_start(slo_t[:], slo_adj[b])
                nc.sync.dma_start(shi_t[:], shim1_adj[b])
                st[b] = {
                    "slo": slo_t, "shi": shi_t,
                    # per-chunk stat partials, indexed [partition, half, chunk]
                    "sum": stats_pool.tile([P, 2, NCH], F32, tag="sum_s",
                                           name=f"sum{b}"),
                    "mx": stats_pool.tile([P, 2, NCH], F32, tag="mx_s",
                                          name=f"mx{b}"),
                    "mn": stats_pool.tile([P, 2, NCH], F32, tag="mn_s",
                                          name=f"mn{b}"),
                    "cc": [],
                }

            def pass1_chunk(b, j):
                r0, c0 = b * P, j * C
                s = st[b]
                tb32 = stage_pool.tile([P, 2, C], F32, tag="tb32",
                                       name=f"tb32_{b}_{j}")
                nc.sync.dma_start(tb32[:, 0, :], top[r0:r0 + P, c0:c0 + C])
                nc.sync.dma_start(tb32[:, 1, :], bot[r0:r0 + P, c0:c0 + C])
                # f32 -> f16 on the Activation engine so every Vector op in
                # this chunk runs in the f16 2x mode
                tb16 = stage_pool.tile([P, 2, C], F16, tag="tb16",
                                       name=f"tb16_{b}_{j}")
                nc.scalar.activation(tb16[:], tb32[:], ACT.Copy)

                # y = min(max(iota, slo), shi-1); mask = (y == iota) is the
                # exact [slo, shi) window indicator.
                y16 = scratch_pool.tile([P, C], F16, tag="y16", name=f"y{b}_{j}")
                nc.vector.tensor_scalar(
                    y16[:], iota_t[:], s["slo"][:, j:j + 1], s["shi"][:, j:j + 1],
                    op0=OP.max, op1=OP.min,
                )
                mask = scratch_pool.tile([P, C], F16, tag="mask",
                                         name=f"mask{b}_{j}")
                nc.vector.tensor_tensor(mask[:], y16[:], iota_t[:], op=OP.is_equal)

                # pair diff on the Pool engine (f32 ins are Pool-legal)
                d16 = scratch_pool.tile([P, C], F16, tag="d16", name=f"d{b}_{j}")
                nc.gpsimd.tensor_tensor(d16[:], tb32[:, 1, :], tb32[:, 0, :],
                                        op=OP.subtract)
                d2 = scratch_pool.tile([P, C], F16, tag="d2", name=f"d2_{b}_{j}")
                nc.vector.tensor_tensor(d2[:], mask[:], d16[:], op=OP.mult)

                cc = pop_pool.tile([P, 2, C], F16, tag="cc", name=f"cc{b}_{j}")
                nc.vector.tensor_tensor(cc[:, 0, :], tb16[:, 0, :], d2[:],
                                        op=OP.add)
                nc.vector.tensor_tensor(cc[:, 1, :], tb16[:, 1, :], d2[:],
                                        op=OP.subtract)
                # max/min via f16 TT ladder (2x mode) + short 1x reduce
                h1 = C // 2
                h2 = C // 4
                for op, dst in ((OP.max, s["mx"]), (OP.min, s["mn"])):
                    l1 = scratch_pool.tile([P, 2, h1], F16, tag="lad1",
                                           name=f"l1_{b}_{j}_{op.value}")
                    nc.vector.tensor_tensor(l1[:], cc[:, :, 0:h1],
                                            cc[:, :, h1:C], op=op)
                    l2 = scratch_pool.tile([P, 2, h2], F16, tag="lad2",
                                           name=f"l2_{b}_{j}_{op.value}")
                    nc.vector.tensor_tensor(l2[:], l1[:, :, 0:h2],
                                            l1[:, :, h2:h1], op=op)
                    nc.vector.tensor_reduce(dst[:, :, j:j + 1], l2[:], axis=X,
                                            op=op)
                # row sums on the Activation engine (junk data out, accum kept)
                junk = scratch_pool.tile([P, C], F16, tag="junk",
                                         name=f"junk{b}_{j}")
                nc.scalar.activation(junk[:], cc[:, 0, :], ACT.Copy,
                                     accum_out=s["sum"][:, 0:1, j])
                nc.scalar.activation(junk[:], cc[:, 1, :], ACT.Copy,
                                     accum_out=s["sum"][:, 1:2, j])
                s["cc"].append(cc)

            def finalize_stats(b):
                s = st[b]
                avg_f = stats_pool.tile([P, 2], F32, tag="avg_f", name=f"avg{b}")
                mx_f = stats_pool.tile([P, 2], F32, tag="mx_f", name=f"mxf{b}")
                mn_f = stats_pool.tile([P, 2], F32, tag="mn_f", name=f"mnf{b}")
                nc.vector.reduce_sum(avg_f[:], s["sum"][:], axis=X)
                nc.vector.tensor_scalar(avg_f[:], avg_f[:], 1.0 / L, None,
                                        op0=OP.mult)
                nc.vector.reduce_max(mx_f[:], s["mx"][:], axis=X)
                nc.vector.tensor_reduce(mn_f[:], s["mn"][:], axis=X, op=OP.min)
                s["avg_f"], s["mx_f"], s["mn_f"] = avg_f, mx_f, mn_f

            halves = (
                (0, un_top, um_top, out_top),
                (1, un_bot, um_bot, out_bot),
            )

            def pass2_half(b, j, h):
                r0, c0 = b * P, j * C
                s = st[b]
                _, un_d, um_d, out_d = halves[h]
                un_t = stream_pool.tile([P, C], F32, tag="un", name=f"un{b}_{j}_{h}")
                um_t = stream_pool.tile([P, C], F32, tag="um", name=f"um{b}_{j}_{h}")
                nc.sync.dma_start(un_t[:], un_d[r0:r0 + P, c0:c0 + C])
                nc.sync.dma_start(um_t[:], um_d[r0:r0 + P, c0:c0 + C])
                # u_noise cast on the Activation engine
                un16 = stream_pool.tile([P, C], F16, tag="un16",
                                        name=f"un16_{b}_{j}_{h}")
                nc.scalar.activation(un16[:], un_t[:], ACT.Copy)
                # mq = (um < rate) * avg   (TS 2x, f32 in -> f16 out)
                mq = p2_pool.tile([P, C], F16, tag="mq", name=f"mq{b}_{j}_{h}")
                nc.vector.tensor_scalar(
                    mq[:], um_t[:], MUTATION_RATE, s["avg_f"][:, h:h + 1],
                    op0=OP.is_lt, op1=OP.mult,
                )
                # qa = mq * u_noise ; cc += qa   (both f16 TT 2x)
                qa = p2_pool.tile([P, C], F16, tag="qa", name=f"qa{b}_{j}_{h}")
                nc.vector.tensor_tensor(qa[:], mq[:], un16[:], op=OP.mult)
                cch = s["cc"][j][:, h, :]
                nc.vector.tensor_tensor(cch, cch, qa[:], op=OP.add)
                # clip to [mn, mx] while widening back to f32
                o32 = out_pool.tile([P, C], F32, tag="o32", name=f"o{b}_{j}_{h}")
                nc.vector.tensor_scalar(
                    o32[:], cch, s["mx_f"][:, h:h + 1], s["mn_f"][:, h:h + 1],
                    op0=OP.min, op1=OP.max,
                )
                nc.sync.dma_start(out_d[r0:r0 + P, c0:c0 + C], o32[:])

            # Software pipeline over blocks: block b's pass 2 interleaves with
            # block b+1's pass 1.
            start_block(0)
            for j in range(NCH):
                pass1_chunk(0, j)
            finalize_stats(0)
            for b in range(BLOCKS):
                nxt = b + 1
                if nxt < BLOCKS:
                    start_block(nxt)
                for j in range(NCH):
                    pass2_half(b, j, 0)
                    pass2_half(b, j, 1)
                    if nxt < BLOCKS:
                        pass1_chunk(nxt, j)
                if nxt < BLOCKS:
                    finalize_stats(nxt)
    nc.finalize()
    return nc


def _get_nc():
    if "nc" not in _NC_CACHE:
        _NC_CACHE["nc"] = _build_program()
    return _NC_CACHE["nc"]


def _prepare_in_maps(pop, start_idx, u_mask, u_noise, seg_len):
    pop = np.asarray(pop, dtype=np.float32)
    u_mask = np.asarray(u_mask, dtype=np.float32)
    u_noise = np.asarray(u_noise, dtype=np.float32)
    s_all = np.asarray(start_idx).astype(np.float32).reshape(HALF)
    seg = float(int(np.asarray(seg_len)))

    iota = np.broadcast_to(
        np.arange(C, dtype=np.float16), (P, C)
    ).copy()

    in_maps = []
    for c in range(NCORES):
        p0 = c * PPC
        s = s_all[p0:p0 + PPC].reshape(BLOCKS, P, 1)
        off = (np.arange(NCH, dtype=np.float32) * C).reshape(1, 1, NCH)
        slo_adj = np.ascontiguousarray(s - off)
        shim1_adj = np.ascontiguousarray(s + seg - 1.0 - off)
        in_maps.append({
            "top": pop[p0:p0 + PPC],
            "bot": pop[HALF + p0:HALF + p0 + PPC],
            "un_top": u_noise[p0:p0 + PPC],
            "un_bot": u_noise[HALF + p0:HALF + p0 + PPC],
            "um_top": u_mask[p0:p0 + PPC],
            "um_bot": u_mask[HALF + p0:HALF + p0 + PPC],
            "slo_adj": slo_adj,
            "shim1_adj": shim1_adj,
            "iota_in": iota,
        })
    return in_maps


def run(pop, start_idx, u_mask, u_noise, seg_len, trace=False):
    """Run on 8 cores; returns (full_output, BassKernelResults)."""
    nc = _get_nc()
    in_maps = _prepare_in_maps(pop, start_idx, u_mask, u_noise, seg_len)
    res = run_bass_kernel_spmd(
        nc, in_maps, core_ids=list(range(NCORES)), trace=trace
    )
    out = np.empty((N, L), dtype=np.float32)
    for c in range(NCORES):
        p0 = c * PPC
        out[p0:p0 + PPC] = res.results[c]["out_top"]
        out[HALF + p0:HALF + p0 + PPC] = res.results[c]["out_bot"]
    return out, res


def kernel(pop, start_idx, u_mask, u_noise, seg_len):
    out, _ = run(pop, start_idx, u_mask, u_noise, seg_len)
    return out


# revision 18
# speedup vs baseline: 1.0332x; 1.0332x over previous
"""Trainium2 Bass kernel for the GeneticAlgorithm step.

Computation (per population pair i, i+N/2):
  crossover: swap cols [s_i, s_i+seg) between the two rows
  stats:     per-row mean / min / max of the crossed matrix
  mutation:  out = where(u_mask < 0.01, clip(crossed + u_noise*avg, mn, mx), crossed)

Key rewrite: since mn <= crossed <= mx per row, clip(crossed, mn, mx) == crossed,
so  out = clip(crossed + (u_mask < 0.01) * u_noise * avg, mn, mx)  exactly.

Sharding: data-parallel over 8 cores; core c owns pairs [256c, 256c+256).
Top/bottom partner rows are co-resident, all reductions are per-row, so there
is no cross-core communication.

Engine plan per [128, 2048] chunk (both halves):
  window mask via integer trick  y = min(max(iota, slo), shi-1);
  mask = (y == iota)  -- exact in fp16 since equality only fires for
  iota in [0, 2047], all fp16-exact.
    DVE:  y (tensor_scalar 4x f16), d2 = mask*d (TT f16 2x),
          crossed halves via STT +/-d2 with free row-sum accums,
          max/min row reduces, and in pass 2 two TS (2x) + two f16 TT (2x)
          for (um<rate)*avg, *u_noise, +crossed, clip/widen.
    POOL: mask equality TT and the f32 pair diff d = bot - top (idle engine).
    ACT:  u_noise f32->f16 casts (idle engine).
fp16 intermediates cost ~3e-3 abs error vs ~5.4 data range; harness gate is
rel 2e-2 (~0.11 abs).
"""

import numpy as np

import concourse.bass as bass
import concourse.bacc as bacc
import concourse.mybir as mybir
from concourse.bass_utils import run_bass_kernel_spmd
from concourse.tile import TileContext

# Problem geometry (hardcoded per spec).
N = 4096           # population size
L = 16384          # genes per individual
HALF = N // 2      # 2048 pairs
NCORES = 8
PPC = HALF // NCORES   # 256 pairs per core
P = 128                # partitions
BLOCKS = PPC // P      # 2 blocks of 128 pairs per core
C = 2048               # column chunk
NCH = L // C           # chunks per row
MUTATION_RATE = 0.01
MASK_ON_GP = False     # Pool ISA check rejects f16 is_equal TT
D_ON_GP = True         # pair diff TT on the Pool engine

F32 = mybir.dt.float32
F16 = mybir.dt.float16
X = mybir.AxisListType.X
OP = mybir.AluOpType
ACT = mybir.ActivationFunctionType

_NC_CACHE = {}


def _build_program():
    nc = bacc.Bacc()

    top = nc.dram_tensor("top", [PPC, L], F32, kind="ExternalInput")
    bot = nc.dram_tensor("bot", [PPC, L], F32, kind="ExternalInput")
    un_top = nc.dram_tensor("un_top", [PPC, L], F32, kind="ExternalInput")
    un_bot = nc.dram_tensor("un_bot", [PPC, L], F32, kind="ExternalInput")
    um_top = nc.dram_tensor("um_top", [PPC, L], F32, kind="ExternalInput")
    um_bot = nc.dram_tensor("um_bot", [PPC, L], F32, kind="ExternalInput")
    # Per-chunk-adjusted crossover bounds: slo_adj[b,p,j] = s - C*j,
    # shim1_adj[b,p,j] = s + seg - 1 - C*j  (f32; exact for values < 2^24).
    slo_adj = nc.dram_tensor("slo_adj", [BLOCKS, P, NCH], F32, kind="ExternalInput")
    shim1_adj = nc.dram_tensor("shim1_adj", [BLOCKS, P, NCH], F32,
                               kind="ExternalInput")
    iota_in = nc.dram_tensor("iota_in", [P, C], F16, kind="ExternalInput")

    out_top = nc.dram_tensor("out_top", [PPC, L], F32, kind="ExternalOutput")
    out_bot = nc.dram_tensor("out_bot", [PPC, L], F32, kind="ExternalOutput")

    with TileContext(nc) as tc:
        with (
            tc.tile_pool(name="const", bufs=1) as const_pool,
            tc.tile_pool(name="popc", bufs=NCH) as pop_pool,
            tc.tile_pool(name="stage", bufs=2) as stage_pool,
            tc.tile_pool(name="scratch", bufs=1) as scratch_pool,
            tc.tile_pool(name="stream", bufs=2) as stream_pool,
            tc.tile_pool(name="p2tmp", bufs=1) as p2_pool,
            tc.tile_pool(name="outp", bufs=2) as out_pool,
            tc.tile_pool(name="stats", bufs=2) as stats_pool,
        ):
            iota_t = const_pool.tile([P, C], F16)
            nc.sync.dma_start(iota_t[:], iota_in[:])

            st = {}  # per-block tile state

            def start_block(b):
                slo_t = stats_pool.tile([P, NCH], F32, tag="slo", name=f"slo{b}")
                shi_t = stats_pool.tile([P, NCH], F32, tag="shi", name=f"shi{b}")
                nc.sync.dma_start(slo_t[:], slo_adj[b])
                nc.sync.dma_start(shi_t[:], shim1_adj[b])
                st[b] = {
                    "slo": slo_t, "shi": shi_t,
                    # per-chunk stat partials, indexed [partition, half, chunk]
                    "sum": stats_pool.tile([P, 2, NCH], F32, tag="sum_s",
                                           name=f"sum{b}"),
                    "mx": stats_pool.tile([P, 2, NCH], F32, tag="mx_s",
                                          name=f"mx{b}"),
                    "mn": stats_pool.tile([P, 2, NCH], F32, tag="mn_s",
                                          name=f"mn{b}"),
                    "cc": [],
                }

            def pass1_chunk(b, j):
                r0, c0 = b * P, j * C
                s = st[b]
                tb32 = stage_pool.tile([P, 2, C], F32, tag="tb32",
                                       name=f"tb32_{b}_{j}")
                nc.sync.dma_start(tb32[:, 0, :], top[r0:r0 + P, c0:c0 + C])
                nc.sync.dma_start(tb32[:, 1, :], bot[r0:r0 + P, c0:c0 + C])
                # f32 -> f16 on the Activation engine so every Vector op in
                # this chunk runs in the f16 2x mode
                tb16 = stage_pool.tile([P, 2, C], F16, tag="tb16",
                                       name=f"tb16_{b}_{j}")
                nc.scalar.activation(tb16[:], tb32[:], ACT.Copy)

                # y = min(max(iota, slo), shi-1); mask = (y == iota) is the
                # exact [slo, shi) window indicator.
                y16 = scratch_pool.tile([P, C], F16, tag="y16", name=f"y{b}_{j}")
                nc.vector.tensor_scalar(
                    y16[:], iota_t[:], s["slo"][:, j:j + 1], s["shi"][:, j:j + 1],
                    op0=OP.max, op1=OP.min,
                )
                mask = scratch_pool.tile([P, C], F16, tag="mask",
                                         name=f"mask{b}_{j}")
                nc.vector.tensor_tensor(mask[:], y16[:], iota_t[:], op=OP.is_equal)

                d16 = scratch_pool.tile([P, C], F16, tag="d16", name=f"d{b}_{j}")
                nc.vector.tensor_tensor(d16[:], tb16[:, 1, :], tb16[:, 0, :],
                                        op=OP.subtract)
                d2 = scratch_pool.tile([P, C], F16, tag="d2", name=f"d2_{b}_{j}")
                nc.vector.tensor_tensor(d2[:], mask[:], d16[:], op=OP.mult)

                cc = pop_pool.tile([P, 2, C], F16, tag="cc", name=f"cc{b}_{j}")
                nc.vector.tensor_tensor(cc[:, 0, :], tb16[:, 0, :], d2[:],
                                        op=OP.add)
                nc.vector.tensor_tensor(cc[:, 1, :], tb16[:, 1, :], d2[:],
                                        op=OP.subtract)
                # max/min via f16 TT ladder (2x mode) + short 1x reduce
                h1 = C // 2
                h2 = C // 4
                for op, dst in ((OP.max, s["mx"]), (OP.min, s["mn"])):
                    l1 = scratch_pool.tile([P, 2, h1], F16, tag="lad1",
                                           name=f"l1_{b}_{j}_{op.value}")
                    nc.vector.tensor_tensor(l1[:], cc[:, :, 0:h1],
                                            cc[:, :, h1:C], op=op)
                    l2 = scratch_pool.tile([P, 2, h2], F16, tag="lad2",
                                           name=f"l2_{b}_{j}_{op.value}")
                    nc.vector.tensor_tensor(l2[:], l1[:, :, 0:h2],
                                            l1[:, :, h2:h1], op=op)
                    nc.vector.tensor_reduce(dst[:, :, j:j + 1], l2[:], axis=X,
                                            op=op)
                # row sums on the Activation engine (junk data out, accum kept)
                junk = scratch_pool.tile([P, C], F16, tag="junk",
                                         name=f"junk{b}_{j}")
                nc.scalar.activation(junk[:], cc[:, 0, :], ACT.Copy,
                                     accum_out=s["sum"][:, 0:1, j])
                nc.scalar.activation(junk[:], cc[:, 1, :], ACT.Copy,
                                     accum_out=s["sum"][:, 1:2, j])
                s["cc"].append(cc)

            def finalize_stats(b):
                s = st[b]
                avg_f = stats_pool.tile([P, 2], F32, tag="avg_f", name=f"avg{b}")
                mx_f = stats_pool.tile([P, 2], F32, tag="mx_f", name=f"mxf{b}")
                mn_f = stats_pool.tile([P, 2], F32, tag="mn_f", name=f"mnf{b}")
                nc.vector.reduce_sum(avg_f[:], s["sum"][:], axis=X)
                nc.vector.tensor_scalar(avg_f[:], avg_f[:], 1.0 / L, None,
                                        op0=OP.mult)
                nc.vector.reduce_max(mx_f[:], s["mx"][:], axis=X)
                nc.vector.tensor_reduce(mn_f[:], s["mn"][:], axis=X, op=OP.min)
                s["avg_f"], s["mx_f"], s["mn_f"] = avg_f, mx_f, mn_f

            halves = (
                (0, un_top, um_top, out_top),
                (1, un_bot, um_bot, out_bot),
            )

            def pass2_half(b, j, h):
                r0, c0 = b * P, j * C
                s = st[b]
                _, un_d, um_d, out_d = halves[h]
                un_t = stream_pool.tile([P, C], F32, tag="un", name=f"un{b}_{j}_{h}")
                um_t = stream_pool.tile([P, C], F32, tag="um", name=f"um{b}_{j}_{h}")
                nc.sync.dma_start(un_t[:], un_d[r0:r0 + P, c0:c0 + C])
                nc.sync.dma_start(um_t[:], um_d[r0:r0 + P, c0:c0 + C])
                # u_noise cast on the Activation engine
                un16 = stream_pool.tile([P, C], F16, tag="un16",
                                        name=f"un16_{b}_{j}_{h}")
                nc.scalar.activation(un16[:], un_t[:], ACT.Copy)
                # mq = (um < rate) * avg   (TS 2x, f32 in -> f16 out)
                mq = p2_pool.tile([P, C], F16, tag="mq", name=f"mq{b}_{j}_{h}")
                nc.vector.tensor_scalar(
                    mq[:], um_t[:], MUTATION_RATE, s["avg_f"][:, h:h + 1],
                    op0=OP.is_lt, op1=OP.mult,
                )
                # qa = mq * u_noise ; cc += qa   (both f16 TT 2x)
                qa = p2_pool.tile([P, C], F16, tag="qa", name=f"qa{b}_{j}_{h}")
                nc.vector.tensor_tensor(qa[:], mq[:], un16[:], op=OP.mult)
                cch = s["cc"][j][:, h, :]
                nc.vector.tensor_tensor(cch, cch, qa[:], op=OP.add)
                # clip to [mn, mx] while widening back to f32
                o32 = out_pool.tile([P, C], F32, tag="o32", name=f"o{b}_{j}_{h}")
                nc.vector.tensor_scalar(
                    o32[:], cch, s["mx_f"][:, h:h + 1], s["mn_f"][:, h:h + 1],
                    op0=OP.min, op1=OP.max,
                )
                nc.sync.dma_start(out_d[r0:r0 + P, c0:c0 + C], o32[:])

            # Software pipeline over blocks: block b's pass 2 interleaves with
            # block b+1's pass 1.
            start_block(0)
            for j in range(NCH):
                pass1_chunk(0, j)
            finalize_stats(0)
            for b in range(BLOCKS):
                nxt = b + 1
                if nxt < BLOCKS:
                    start_block(nxt)
                for j in range(NCH):
                    pass2_half(b, j, 0)
                    pass2_half(b, j, 1)
                    if nxt < BLOCKS:
                        pass1_chunk(nxt, j)
                if nxt < BLOCKS:
                    finalize_stats(nxt)
    nc.finalize()
    return nc


def _get_nc():
    if "nc" not in _NC_CACHE:
        _NC_CACHE["nc"] = _build_program()
    return _NC_CACHE["nc"]


def _prepare_in_maps(pop, start_idx, u_mask, u_noise, seg_len):
    pop = np.asarray(pop, dtype=np.float32)
    u_mask = np.asarray(u_mask, dtype=np.float32)
    u_noise = np.asarray(u_noise, dtype=np.float32)
    s_all = np.asarray(start_idx).astype(np.float32).reshape(HALF)
    seg = float(int(np.asarray(seg_len)))

    iota = np.broadcast_to(
        np.arange(C, dtype=np.float16), (P, C)
    ).copy()

    in_maps = []
    for c in range(NCORES):
        p0 = c * PPC
        s = s_all[p0:p0 + PPC].reshape(BLOCKS, P, 1)
        off = (np.arange(NCH, dtype=np.float32) * C).reshape(1, 1, NCH)
        slo_adj = np.ascontiguousarray(s - off)
        shim1_adj = np.ascontiguousarray(s + seg - 1.0 - off)
        in_maps.append({
            "top": pop[p0:p0 + PPC],
            "bot": pop[HALF + p0:HALF + p0 + PPC],
            "un_top": u_noise[p0:p0 + PPC],
            "un_bot": u_noise[HALF + p0:HALF + p0 + PPC],
            "um_top": u_mask[p0:p0 + PPC],
            "um_bot": u_mask[HALF + p0:HALF + p0 + PPC],
            "slo_adj": slo_adj,
            "shim1_adj": shim1_adj,
            "iota_in": iota,
        })
    return in_maps


def run(pop, start_idx, u_mask, u_noise, seg_len, trace=False):
    """Run on 8 cores; returns (full_output, BassKernelResults)."""
    nc = _get_nc()
    in_maps = _prepare_in_maps(pop, start_idx, u_mask, u_noise, seg_len)
    res = run_bass_kernel_spmd(
        nc, in_maps, core_ids=list(range(NCORES)), trace=trace
    )
    out = np.empty((N, L), dtype=np.float32)
    for c in range(NCORES):
        p0 = c * PPC
        out[p0:p0 + PPC] = res.results[c]["out_top"]
        out[HALF + p0:HALF + p0 + PPC] = res.results[c]["out_bot"]
    return out, res


def kernel(pop, start_idx, u_mask, u_noise, seg_len):
    out, _ = run(pop, start_idx, u_mask, u_noise, seg_len)
    return out


# revision 23
# speedup vs baseline: 1.0753x; 1.0407x over previous
"""Trainium2 Bass kernel for the GeneticAlgorithm step.

Computation (per population pair i, i+N/2):
  crossover: swap cols [s_i, s_i+seg) between the two rows
  stats:     per-row mean / min / max of the crossed matrix
  mutation:  out = where(u_mask < 0.01, clip(crossed + u_noise*avg, mn, mx), crossed)

Key rewrite: since mn <= crossed <= mx per row, clip(crossed, mn, mx) == crossed,
so  out = clip(crossed + (u_mask < 0.01) * u_noise * avg, mn, mx)  exactly.

Sharding: data-parallel over 8 cores; core c owns pairs [256c, 256c+256).
Top/bottom partner rows are co-resident, all reductions are per-row, so there
is no cross-core communication.

Engine plan per [128, 2048] chunk (both halves):
  window mask via integer trick  y = min(max(iota, slo), shi-1);
  mask = (y == iota)  -- exact in fp16 since equality only fires for
  iota in [0, 2047], all fp16-exact.
    DVE:  y (tensor_scalar 4x f16), d2 = mask*d (TT f16 2x),
          crossed halves via STT +/-d2 with free row-sum accums,
          max/min row reduces, and in pass 2 two TS (2x) + two f16 TT (2x)
          for (um<rate)*avg, *u_noise, +crossed, clip/widen.
    POOL: mask equality TT and the f32 pair diff d = bot - top (idle engine).
    ACT:  u_noise f32->f16 casts (idle engine).
fp16 intermediates cost ~3e-3 abs error vs ~5.4 data range; harness gate is
rel 2e-2 (~0.11 abs).
"""

import numpy as np

import concourse.bass as bass
import concourse.bacc as bacc
import concourse.mybir as mybir
from concourse.bass_utils import run_bass_kernel_spmd
from concourse.tile import TileContext

# Problem geometry (hardcoded per spec).
N = 4096           # population size
L = 16384          # genes per individual
HALF = N // 2      # 2048 pairs
NCORES = 8
PPC = HALF // NCORES   # 256 pairs per core
P = 128                # partitions
BLOCKS = PPC // P      # 2 blocks of 128 pairs per core
C = 2048               # column chunk
NCH = L // C           # chunks per row
MUTATION_RATE = 0.01
MASK_ON_GP = False     # Pool ISA check rejects f16 is_equal TT
D_ON_GP = True         # pair diff TT on the Pool engine

F32 = mybir.dt.float32
F16 = mybir.dt.float16
X = mybir.AxisListType.X
OP = mybir.AluOpType
ACT = mybir.ActivationFunctionType

_NC_CACHE = {}


def _build_program():
    nc = bacc.Bacc()

    top = nc.dram_tensor("top", [PPC, L], F32, kind="ExternalInput")
    bot = nc.dram_tensor("bot", [PPC, L], F32, kind="ExternalInput")
    un_top = nc.dram_tensor("un_top", [PPC, L], F32, kind="ExternalInput")
    un_bot = nc.dram_tensor("un_bot", [PPC, L], F32, kind="ExternalInput")
    um_top = nc.dram_tensor("um_top", [PPC, L], F32, kind="ExternalInput")
    um_bot = nc.dram_tensor("um_bot", [PPC, L], F32, kind="ExternalInput")
    # Per-chunk-adjusted crossover bounds: slo_adj[b,p,j] = s - C*j,
    # shim1_adj[b,p,j] = s + seg - 1 - C*j  (f32; exact for values < 2^24).
    slo_adj = nc.dram_tensor("slo_adj", [BLOCKS, P, NCH], F32, kind="ExternalInput")
    shim1_adj = nc.dram_tensor("shim1_adj", [BLOCKS, P, NCH], F32,
                               kind="ExternalInput")
    iota_in = nc.dram_tensor("iota_in", [P, C], F16, kind="ExternalInput")

    out_top = nc.dram_tensor("out_top", [PPC, L], F32, kind="ExternalOutput")
    out_bot = nc.dram_tensor("out_bot", [PPC, L], F32, kind="ExternalOutput")

    with TileContext(nc) as tc:
        with (
            tc.tile_pool(name="const", bufs=1) as const_pool,
            tc.tile_pool(name="popc", bufs=NCH) as pop_pool,
            tc.tile_pool(name="stage", bufs=2) as stage_pool,
            tc.tile_pool(name="scratch", bufs=1) as scratch_pool,
            tc.tile_pool(name="stream", bufs=3) as stream_pool,
            tc.tile_pool(name="p2tmp", bufs=1) as p2_pool,
            tc.tile_pool(name="unp", bufs=2) as un_pool,
            tc.tile_pool(name="outp", bufs=2) as out_pool,
            tc.tile_pool(name="stats", bufs=2) as stats_pool,
        ):
            iota_t = const_pool.tile([P, C], F16)
            nc.sync.dma_start(iota_t[:], iota_in[:])

            st = {}  # per-block tile state

            def start_block(b):
                slo_t = stats_pool.tile([P, NCH], F32, tag="slo", name=f"slo{b}")
                shi_t = stats_pool.tile([P, NCH], F32, tag="shi", name=f"shi{b}")
                nc.sync.dma_start(slo_t[:], slo_adj[b])
                nc.sync.dma_start(shi_t[:], shim1_adj[b])
                st[b] = {
                    "slo": slo_t, "shi": shi_t,
                    # per-chunk stat partials, indexed [partition, half, chunk]
                    "sum": stats_pool.tile([P, 2, NCH], F32, tag="sum_s",
                                           name=f"sum{b}"),
                    "mx": stats_pool.tile([P, 2, NCH], F32, tag="mx_s",
                                          name=f"mx{b}"),
                    "mn": stats_pool.tile([P, 2, NCH], F32, tag="mn_s",
                                          name=f"mn{b}"),
                    "cc": [],
                }

            def pass1_chunk(b, j):
                r0, c0 = b * P, j * C
                s = st[b]
                tb32 = stage_pool.tile([P, 2, C], F32, tag="tb32",
                                       name=f"tb32_{b}_{j}")
                nc.sync.dma_start(tb32[:, 0, :], top[r0:r0 + P, c0:c0 + C])
                nc.sync.dma_start(tb32[:, 1, :], bot[r0:r0 + P, c0:c0 + C])
                # f32 -> f16 cast on the Activation engine, straight into the
                # cc tile (updated in place below) so every Vector op in this
                # chunk runs in the f16 2x mode
                cc = pop_pool.tile([P, 2, C], F16, tag="cc", name=f"cc{b}_{j}")
                nc.scalar.activation(cc[:], tb32[:], ACT.Copy)

                # y = min(max(iota, slo), shi-1); mask = (y == iota) is the
                # exact [slo, shi) window indicator.
                y16 = scratch_pool.tile([P, C], F16, tag="y16", name=f"y{b}_{j}")
                nc.vector.tensor_scalar(
                    y16[:], iota_t[:], s["slo"][:, j:j + 1], s["shi"][:, j:j + 1],
                    op0=OP.max, op1=OP.min,
                )
                mask = scratch_pool.tile([P, C], F16, tag="mask",
                                         name=f"mask{b}_{j}")
                nc.vector.tensor_tensor(mask[:], y16[:], iota_t[:], op=OP.is_equal)

                d16 = scratch_pool.tile([P, C], F16, tag="d16", name=f"d{b}_{j}")
                nc.vector.tensor_tensor(d16[:], cc[:, 1, :], cc[:, 0, :],
                                        op=OP.subtract)
                d2 = scratch_pool.tile([P, C], F16, tag="d2", name=f"d2_{b}_{j}")
                nc.vector.tensor_tensor(d2[:], mask[:], d16[:], op=OP.mult)

                nc.vector.tensor_tensor(cc[:, 0, :], cc[:, 0, :], d2[:],
                                        op=OP.add)
                nc.vector.tensor_tensor(cc[:, 1, :], cc[:, 1, :], d2[:],
                                        op=OP.subtract)
                # max/min via f16 TT ladder (2x mode) + short 1x reduce
                h1 = C // 2
                h2 = C // 4
                for op, dst in ((OP.max, s["mx"]), (OP.min, s["mn"])):
                    l1 = scratch_pool.tile([P, 2, h1], F16, tag="lad1",
                                           name=f"l1_{b}_{j}_{op.value}")
                    nc.vector.tensor_tensor(l1[:], cc[:, :, 0:h1],
                                            cc[:, :, h1:C], op=op)
                    l2 = scratch_pool.tile([P, 2, h2], F16, tag="lad2",
                                           name=f"l2_{b}_{j}_{op.value}")
                    nc.vector.tensor_tensor(l2[:], l1[:, :, 0:h2],
                                            l1[:, :, h2:h1], op=op)
                    nc.vector.tensor_reduce(dst[:, :, j:j + 1], l2[:], axis=X,
                                            op=op)
                # row sums on the Activation engine (junk data out, accum kept)
                junk = scratch_pool.tile([P, C], F16, tag="junk",
                                         name=f"junk{b}_{j}")
                nc.scalar.activation(junk[:], cc[:, 0, :], ACT.Copy,
                                     accum_out=s["sum"][:, 0:1, j])
                nc.scalar.activation(junk[:], cc[:, 1, :], ACT.Copy,
                                     accum_out=s["sum"][:, 1:2, j])
                s["cc"].append(cc)

            def finalize_stats(b):
                s = st[b]
                avg_f = stats_pool.tile([P, 2], F32, tag="avg_f", name=f"avg{b}")
                mx_f = stats_pool.tile([P, 2], F32, tag="mx_f", name=f"mxf{b}")
                mn_f = stats_pool.tile([P, 2], F32, tag="mn_f", name=f"mnf{b}")
                nc.vector.reduce_sum(avg_f[:], s["sum"][:], axis=X)
                nc.vector.tensor_scalar(avg_f[:], avg_f[:], 1.0 / L, None,
                                        op0=OP.mult)
                nc.vector.reduce_max(mx_f[:], s["mx"][:], axis=X)
                nc.vector.tensor_reduce(mn_f[:], s["mn"][:], axis=X, op=OP.min)
                s["avg_f"], s["mx_f"], s["mn_f"] = avg_f, mx_f, mn_f

            halves = (
                (0, un_top, um_top, out_top),
                (1, un_bot, um_bot, out_bot),
            )

            def pass2_half(b, j, h):
                r0, c0 = b * P, j * C
                s = st[b]
                _, un_d, um_d, out_d = halves[h]
                un_t = stream_pool.tile([P, C], F32, tag="un", name=f"un{b}_{j}_{h}")
                um_t = stream_pool.tile([P, C], F32, tag="um", name=f"um{b}_{j}_{h}")
                nc.sync.dma_start(un_t[:], un_d[r0:r0 + P, c0:c0 + C])
                nc.sync.dma_start(um_t[:], um_d[r0:r0 + P, c0:c0 + C])
                # u_noise cast on the Activation engine
                un16 = un_pool.tile([P, C], F16, tag="un16",
                                    name=f"un16_{b}_{j}_{h}")
                nc.scalar.activation(un16[:], un_t[:], ACT.Copy)
                # mq = (um < rate) * avg   (TS 2x, f32 in -> f16 out)
                mq = p2_pool.tile([P, C], F16, tag="mq", name=f"mq{b}_{j}_{h}")
                nc.vector.tensor_scalar(
                    mq[:], um_t[:], MUTATION_RATE, s["avg_f"][:, h:h + 1],
                    op0=OP.is_lt, op1=OP.mult,
                )
                # qa = mq * u_noise ; cc += qa   (both f16 TT 2x)
                qa = p2_pool.tile([P, C], F16, tag="qa", name=f"qa{b}_{j}_{h}")
                nc.vector.tensor_tensor(qa[:], mq[:], un16[:], op=OP.mult)
                cch = s["cc"][j][:, h, :]
                nc.vector.tensor_tensor(cch, cch, qa[:], op=OP.add)
                # clip to [mn, mx] while widening back to f32
                o32 = out_pool.tile([P, C], F32, tag="o32", name=f"o{b}_{j}_{h}")
                nc.vector.tensor_scalar(
                    o32[:], cch, s["mx_f"][:, h:h + 1], s["mn_f"][:, h:h + 1],
                    op0=OP.min, op1=OP.max,
                )
                nc.sync.dma_start(out_d[r0:r0 + P, c0:c0 + C], o32[:])

            # Software pipeline over blocks: block b's pass 2 interleaves with
            # block b+1's pass 1.
            start_block(0)
            for j in range(NCH):
                pass1_chunk(0, j)
            finalize_stats(0)
            for b in range(BLOCKS):
                nxt = b + 1
                if nxt < BLOCKS:
                    start_block(nxt)
                for j in range(NCH):
                    pass2_half(b, j, 0)
                    pass2_half(b, j, 1)
                    if nxt < BLOCKS:
                        pass1_chunk(nxt, j)
                if nxt < BLOCKS:
                    finalize_stats(nxt)
    nc.finalize()
    return nc


def _get_nc():
    if "nc" not in _NC_CACHE:
        _NC_CACHE["nc"] = _build_program()
    return _NC_CACHE["nc"]


def _prepare_in_maps(pop, start_idx, u_mask, u_noise, seg_len):
    pop = np.asarray(pop, dtype=np.float32)
    u_mask = np.asarray(u_mask, dtype=np.float32)
    u_noise = np.asarray(u_noise, dtype=np.float32)
    s_all = np.asarray(start_idx).astype(np.float32).reshape(HALF)
    seg = float(int(np.asarray(seg_len)))

    iota = np.broadcast_to(
        np.arange(C, dtype=np.float16), (P, C)
    ).copy()

    in_maps = []
    for c in range(NCORES):
        p0 = c * PPC
        s = s_all[p0:p0 + PPC].reshape(BLOCKS, P, 1)
        off = (np.arange(NCH, dtype=np.float32) * C).reshape(1, 1, NCH)
        slo_adj = np.ascontiguousarray(s - off)
        shim1_adj = np.ascontiguousarray(s + seg - 1.0 - off)
        in_maps.append({
            "top": pop[p0:p0 + PPC],
            "bot": pop[HALF + p0:HALF + p0 + PPC],
            "un_top": u_noise[p0:p0 + PPC],
            "un_bot": u_noise[HALF + p0:HALF + p0 + PPC],
            "um_top": u_mask[p0:p0 + PPC],
            "um_bot": u_mask[HALF + p0:HALF + p0 + PPC],
            "slo_adj": slo_adj,
            "shim1_adj": shim1_adj,
            "iota_in": iota,
        })
    return in_maps


def run(pop, start_idx, u_mask, u_noise, seg_len, trace=False):
    """Run on 8 cores; returns (full_output, BassKernelResults)."""
    nc = _get_nc()
    in_maps = _prepare_in_maps(pop, start_idx, u_mask, u_noise, seg_len)
    res = run_bass_kernel_spmd(
        nc, in_maps, core_ids=list(range(NCORES)), trace=trace
    )
    out = np.empty((N, L), dtype=np.float32)
    for c in range(NCORES):
        p0 = c * PPC
        out[p0:p0 + PPC] = res.results[c]["out_top"]
        out[HALF + p0:HALF + p0 + PPC] = res.results[c]["out_bot"]
    return out, res


def kernel(pop, start_idx, u_mask, u_noise, seg_len):
    out, _ = run(pop, start_idx, u_mask, u_noise, seg_len)
    return out


# revision 25
# speedup vs baseline: 1.1089x; 1.0313x over previous
"""Trainium2 Bass kernel for the GeneticAlgorithm step.

Computation (per population pair i, i+N/2):
  crossover: swap cols [s_i, s_i+seg) between the two rows
  stats:     per-row mean / min / max of the crossed matrix
  mutation:  out = where(u_mask < 0.01, clip(crossed + u_noise*avg, mn, mx), crossed)

Key rewrite: since mn <= crossed <= mx per row, clip(crossed, mn, mx) == crossed,
so  out = clip(crossed + (u_mask < 0.01) * u_noise * avg, mn, mx)  exactly.

Sharding: data-parallel over 8 cores; core c owns pairs [256c, 256c+256).
Top/bottom partner rows are co-resident, all reductions are per-row, so there
is no cross-core communication.

Engine plan per [128, 2048] chunk (both halves):
  window mask via integer trick  y = min(max(iota, slo), shi-1);
  mask = (y == iota)  -- exact in fp16 since equality only fires for
  iota in [0, 2047], all fp16-exact.
    DVE:  y (tensor_scalar f16 4x mode), mask/diff/crossed updates as all-f16
          tensor_tensor ops (2x mode), max/min row stats via a f16 TT ladder
          (2048->1024->512) + short 1x reduce, and in pass 2 two TS (2x) +
          two f16 TT (2x) for (um<rate)*avg, *u_noise, +crossed, clip/widen.
    ACT:  pop f32->f16 casts (straight into the cc tile), u_noise casts, and
          row sums via Copy+accum_out (otherwise idle engine).
fp16 intermediates cost ~5e-3 abs error vs ~5.4 data range; harness gate is
rel 2e-2 (~0.11 abs). The kernel ends DMA-bound: 135 MB/core at the chip
HBM ceiling (~2.5 TB/s across 8 cores) is ~435 us of pure transfer.
"""

import numpy as np

import concourse.bass as bass
import concourse.bacc as bacc
import concourse.mybir as mybir
from concourse.bass_utils import run_bass_kernel_spmd
from concourse.tile import TileContext

# Problem geometry (hardcoded per spec).
N = 4096           # population size
L = 16384          # genes per individual
HALF = N // 2      # 2048 pairs
NCORES = 8
PPC = HALF // NCORES   # 256 pairs per core
P = 128                # partitions
BLOCKS = PPC // P      # 2 blocks of 128 pairs per core
C = 2048               # column chunk
NCH = L // C           # chunks per row
MUTATION_RATE = 0.01

F32 = mybir.dt.float32
F16 = mybir.dt.float16
X = mybir.AxisListType.X
OP = mybir.AluOpType
ACT = mybir.ActivationFunctionType

_NC_CACHE = {}


def _build_program():
    nc = bacc.Bacc()

    top = nc.dram_tensor("top", [PPC, L], F32, kind="ExternalInput")
    bot = nc.dram_tensor("bot", [PPC, L], F32, kind="ExternalInput")
    un_top = nc.dram_tensor("un_top", [PPC, L], F32, kind="ExternalInput")
    un_bot = nc.dram_tensor("un_bot", [PPC, L], F32, kind="ExternalInput")
    um_top = nc.dram_tensor("um_top", [PPC, L], F32, kind="ExternalInput")
    um_bot = nc.dram_tensor("um_bot", [PPC, L], F32, kind="ExternalInput")
    # Per-chunk-adjusted crossover bounds: slo_adj[b,p,j] = s - C*j,
    # shim1_adj[b,p,j] = s + seg - 1 - C*j  (f32; exact for values < 2^24).
    slo_adj = nc.dram_tensor("slo_adj", [BLOCKS, P, NCH], F32, kind="ExternalInput")
    shim1_adj = nc.dram_tensor("shim1_adj", [BLOCKS, P, NCH], F32,
                               kind="ExternalInput")
    iota_in = nc.dram_tensor("iota_in", [P, C], F16, kind="ExternalInput")

    out_top = nc.dram_tensor("out_top", [PPC, L], F32, kind="ExternalOutput")
    out_bot = nc.dram_tensor("out_bot", [PPC, L], F32, kind="ExternalOutput")

    with TileContext(nc) as tc:
        with (
            tc.tile_pool(name="const", bufs=1) as const_pool,
            tc.tile_pool(name="popc", bufs=NCH) as pop_pool,
            tc.tile_pool(name="stage", bufs=2) as stage_pool,
            tc.tile_pool(name="scratch", bufs=1) as scratch_pool,
            tc.tile_pool(name="stream", bufs=3) as stream_pool,
            tc.tile_pool(name="p2tmp", bufs=1) as p2_pool,
            tc.tile_pool(name="unp", bufs=2) as un_pool,
            tc.tile_pool(name="outp", bufs=2) as out_pool,
            tc.tile_pool(name="stats", bufs=2) as stats_pool,
        ):
            iota_t = const_pool.tile([P, C], F16)
            nc.sync.dma_start(iota_t[:], iota_in[:])

            st = {}  # per-block tile state

            def start_block(b):
                slo_t = stats_pool.tile([P, NCH], F32, tag="slo", name=f"slo{b}")
                shi_t = stats_pool.tile([P, NCH], F32, tag="shi", name=f"shi{b}")
                nc.sync.dma_start(slo_t[:], slo_adj[b])
                nc.sync.dma_start(shi_t[:], shim1_adj[b])
                st[b] = {
                    "slo": slo_t, "shi": shi_t,
                    # per-chunk stat partials, indexed [partition, half, chunk]
                    "sum": stats_pool.tile([P, 2, NCH], F32, tag="sum_s",
                                           name=f"sum{b}"),
                    "mx": stats_pool.tile([P, 2, NCH], F32, tag="mx_s",
                                          name=f"mx{b}"),
                    "mn": stats_pool.tile([P, 2, NCH], F32, tag="mn_s",
                                          name=f"mn{b}"),
                    "cc": [],
                }

            def pass1_chunk(b, j):
                r0, c0 = b * P, j * C
                s = st[b]
                tb32 = stage_pool.tile([P, 2, C], F32, tag="tb32",
                                       name=f"tb32_{b}_{j}")
                nc.sync.dma_start(tb32[:, 0, :], top[r0:r0 + P, c0:c0 + C])
                nc.sync.dma_start(tb32[:, 1, :], bot[r0:r0 + P, c0:c0 + C])
                # f32 -> f16 cast on the Activation engine, straight into the
                # cc tile (updated in place below) so every Vector op in this
                # chunk runs in the f16 2x mode
                cc = pop_pool.tile([P, 2, C], F16, tag="cc", name=f"cc{b}_{j}")
                nc.scalar.activation(cc[:], tb32[:], ACT.Copy)

                # y = min(max(iota, slo), shi-1); mask = (y == iota) is the
                # exact [slo, shi) window indicator.
                y16 = scratch_pool.tile([P, C], F16, tag="y16", name=f"y{b}_{j}")
                nc.vector.tensor_scalar(
                    y16[:], iota_t[:], s["slo"][:, j:j + 1], s["shi"][:, j:j + 1],
                    op0=OP.max, op1=OP.min,
                )
                mask = scratch_pool.tile([P, C], F16, tag="mask",
                                         name=f"mask{b}_{j}")
                nc.vector.tensor_tensor(mask[:], y16[:], iota_t[:], op=OP.is_equal)

                d16 = scratch_pool.tile([P, C], F16, tag="d16", name=f"d{b}_{j}")
                nc.vector.tensor_tensor(d16[:], cc[:, 1, :], cc[:, 0, :],
                                        op=OP.subtract)
                d2 = scratch_pool.tile([P, C], F16, tag="d2", name=f"d2_{b}_{j}")
                nc.vector.tensor_tensor(d2[:], mask[:], d16[:], op=OP.mult)

                nc.vector.tensor_tensor(cc[:, 0, :], cc[:, 0, :], d2[:],
                                        op=OP.add)
                nc.vector.tensor_tensor(cc[:, 1, :], cc[:, 1, :], d2[:],
                                        op=OP.subtract)
                # max/min via f16 TT ladder (2x mode) + short 1x reduce
                h1 = C // 2
                h2 = C // 4
                for op, dst in ((OP.max, s["mx"]), (OP.min, s["mn"])):
                    l1 = scratch_pool.tile([P, 2, h1], F16, tag="lad1",
                                           name=f"l1_{b}_{j}_{op.value}")
                    nc.vector.tensor_tensor(l1[:], cc[:, :, 0:h1],
                                            cc[:, :, h1:C], op=op)
                    l2 = scratch_pool.tile([P, 2, h2], F16, tag="lad2",
                                           name=f"l2_{b}_{j}_{op.value}")
                    nc.vector.tensor_tensor(l2[:], l1[:, :, 0:h2],
                                            l1[:, :, h2:h1], op=op)
                    nc.vector.tensor_reduce(dst[:, :, j:j + 1], l2[:], axis=X,
                                            op=op)
                # row sums on the Activation engine (junk data out, accum kept)
                junk = scratch_pool.tile([P, C], F16, tag="junk",
                                         name=f"junk{b}_{j}")
                nc.scalar.activation(junk[:], cc[:, 0, :], ACT.Copy,
                                     accum_out=s["sum"][:, 0:1, j])
                nc.scalar.activation(junk[:], cc[:, 1, :], ACT.Copy,
                                     accum_out=s["sum"][:, 1:2, j])
                s["cc"].append(cc)

            def finalize_stats(b):
                s = st[b]
                avg_f = stats_pool.tile([P, 2], F32, tag="avg_f", name=f"avg{b}")
                mx_f = stats_pool.tile([P, 2], F32, tag="mx_f", name=f"mxf{b}")
                mn_f = stats_pool.tile([P, 2], F32, tag="mn_f", name=f"mnf{b}")
                nc.vector.reduce_sum(avg_f[:], s["sum"][:], axis=X)
                nc.vector.tensor_scalar(avg_f[:], avg_f[:], 1.0 / L, None,
                                        op0=OP.mult)
                nc.vector.reduce_max(mx_f[:], s["mx"][:], axis=X)
                nc.vector.tensor_reduce(mn_f[:], s["mn"][:], axis=X, op=OP.min)
                s["avg_f"], s["mx_f"], s["mn_f"] = avg_f, mx_f, mn_f

            halves = (
                (0, un_top, um_top, out_top),
                (1, un_bot, um_bot, out_bot),
            )

            def pass2_half(b, j, h):
                r0, c0 = b * P, j * C
                s = st[b]
                _, un_d, um_d, out_d = halves[h]
                un_t = stream_pool.tile([P, C], F32, tag="un", name=f"un{b}_{j}_{h}")
                um_t = stream_pool.tile([P, C], F32, tag="um", name=f"um{b}_{j}_{h}")
                nc.sync.dma_start(un_t[:], un_d[r0:r0 + P, c0:c0 + C])
                nc.sync.dma_start(um_t[:], um_d[r0:r0 + P, c0:c0 + C])
                # u_noise cast on the Activation engine
                un16 = un_pool.tile([P, C], F16, tag="un16",
                                    name=f"un16_{b}_{j}_{h}")
                nc.scalar.activation(un16[:], un_t[:], ACT.Copy)
                # mq = (um < rate) * avg   (TS 2x, f32 in -> f16 out)
                mq = p2_pool.tile([P, C], F16, tag="mq", name=f"mq{b}_{j}_{h}")
                nc.vector.tensor_scalar(
                    mq[:], um_t[:], MUTATION_RATE, s["avg_f"][:, h:h + 1],
                    op0=OP.is_lt, op1=OP.mult,
                )
                # qa = mq * u_noise ; cc += qa   (both f16 TT 2x)
                qa = p2_pool.tile([P, C], F16, tag="qa", name=f"qa{b}_{j}_{h}")
                nc.vector.tensor_tensor(qa[:], mq[:], un16[:], op=OP.mult)
                cch = s["cc"][j][:, h, :]
                nc.vector.tensor_tensor(cch, cch, qa[:], op=OP.add)
                # clip to [mn, mx] while widening back to f32
                o32 = out_pool.tile([P, C], F32, tag="o32", name=f"o{b}_{j}_{h}")
                nc.vector.tensor_scalar(
                    o32[:], cch, s["mx_f"][:, h:h + 1], s["mn_f"][:, h:h + 1],
                    op0=OP.min, op1=OP.max,
                )
                nc.sync.dma_start(out_d[r0:r0 + P, c0:c0 + C], o32[:])

            # Software pipeline over blocks: block b's pass 2 interleaves with
            # block b+1's pass 1.
            start_block(0)
            for j in range(NCH):
                pass1_chunk(0, j)
            finalize_stats(0)
            for b in range(BLOCKS):
                nxt = b + 1
                if nxt < BLOCKS:
                    start_block(nxt)
                for j in range(NCH):
                    pass2_half(b, j, 0)
                    pass2_half(b, j, 1)
                    if nxt < BLOCKS:
                        pass1_chunk(nxt, j)
                if nxt < BLOCKS:
                    finalize_stats(nxt)
    nc.finalize()
    return nc


def _get_nc():
    if "nc" not in _NC_CACHE:
        _NC_CACHE["nc"] = _build_program()
    return _NC_CACHE["nc"]


def _prepare_in_maps(pop, start_idx, u_mask, u_noise, seg_len):
    pop = np.asarray(pop, dtype=np.float32)
    u_mask = np.asarray(u_mask, dtype=np.float32)
    u_noise = np.asarray(u_noise, dtype=np.float32)
    s_all = np.asarray(start_idx).astype(np.float32).reshape(HALF)
    seg = float(int(np.asarray(seg_len)))

    iota = np.broadcast_to(
        np.arange(C, dtype=np.float16), (P, C)
    ).copy()

    in_maps = []
    for c in range(NCORES):
        p0 = c * PPC
        s = s_all[p0:p0 + PPC].reshape(BLOCKS, P, 1)
        off = (np.arange(NCH, dtype=np.float32) * C).reshape(1, 1, NCH)
        slo_adj = np.ascontiguousarray(s - off)
        shim1_adj = np.ascontiguousarray(s + seg - 1.0 - off)
        in_maps.append({
            "top": pop[p0:p0 + PPC],
            "bot": pop[HALF + p0:HALF + p0 + PPC],
            "un_top": u_noise[p0:p0 + PPC],
            "un_bot": u_noise[HALF + p0:HALF + p0 + PPC],
            "um_top": u_mask[p0:p0 + PPC],
            "um_bot": u_mask[HALF + p0:HALF + p0 + PPC],
            "slo_adj": slo_adj,
            "shim1_adj": shim1_adj,
            "iota_in": iota,
        })
    return in_maps


def run(pop, start_idx, u_mask, u_noise, seg_len, trace=False):
    """Run on 8 cores; returns (full_output, BassKernelResults)."""
    nc = _get_nc()
    in_maps = _prepare_in_maps(pop, start_idx, u_mask, u_noise, seg_len)
    res = run_bass_kernel_spmd(
        nc, in_maps, core_ids=list(range(NCORES)), trace=trace
    )
    out = np.empty((N, L), dtype=np.float32)
    for c in range(NCORES):
        p0 = c * PPC
        out[p0:p0 + PPC] = res.results[c]["out_top"]
        out[HALF + p0:HALF + p0 + PPC] = res.results[c]["out_bot"]
    return out, res


def kernel(pop, start_idx, u_mask, u_noise, seg_len):
    out, _ = run(pop, start_idx, u_mask, u_noise, seg_len)
    return out
